# revision 1
# baseline (speedup 1.0000x reference)
"""Qwen2.5-VL attention (mrope + GQA + causal mask + o_proj) on 8 Trainium2
NeuronCores.

Sharding: batch x query-strip-pair, causal-balanced. Core c handles batch
b = c//4 and the two 256-row query strips j=c%4 and 7-j of that batch
(rows [256j, 256j+256) and [256(7-j), 256(7-j)+256)). Causality makes the
key ranges uniform across cores: strip j only needs keys [0, 256(j+1)) --
always within the first 1024 keys -- and strip 7-j needs at most all 2048.
So the (identical) per-core program runs attention for the A-slot (strip j)
over key tiles 0..7 only and for the B-slot (strip 7-j) over all 16 key
tiles, with the host-supplied exp(mask) data handling the causal boundary.
This cuts scores/stats/PV PE work by 25% vs the full-keys version while
keeping a single SPMD program. Each core computes K/V projections for all
2048 tokens of its batch, Q projection + o_proj for its 512 query rows, and
writes a [512, 2048] output slice. Host scatters rows back - no cross-core
reduction.

On-device layout: everything transposed so the PE contraction dim is always
on partitions.  Host pre-transposes hidden (xT), weights (wqT/wkvT/woT),
merged-mrope cos/sin, and the mask tiles (pre-packed in SBUF layout).
  - QT/KT produced as [d, t]; scores computed transposed S^T[k, q]
  - key tiles 0..7 (phase 1): per head, scores for both slots at once
    (N=512); the A-slot columns get the causal exp(mask) multiply, the
    B-slot columns are never masked there (B rows >= 1024 > keys 0..1023)
  - key tiles 8..15 (phase 2): per head-PAIR, scores for the two heads'
    B-slots side by side (N=512); exp(mask) multiply with host-duplicated
    mask data
  - exp on ScalarE straight from PSUM with the 1/sqrt(D) scale folded in
  - softmax denominators via ones[128,128] matmuls (sums arrive broadcast
    across partitions); phase-2 sums/outputs are combined with phase 1 on
    VectorE, then reciprocal + multiply
  - PV accumulates outT[d, q]; o_proj consumes outT directly as lhsT
  - Q projection is interleaved with attention per head group; the wq
    stream and the xq load ride the Scalar engine's DMA queue so they never
    queue behind the Sync engine's x/wkv/mask traffic

Dtypes ("mix" mode): the Q/K path (x, wq/wk/wv, cos/sin, q, k, scores) runs
fp32r -- scores reach |s|~22, and bf16's 0.4% steps there would perturb
softmax weights by ~10% (measured 1.2e-2 rel err end to end); fp32r keeps
that path near-exact. The exp outputs, V, and the o_proj operands run bf16,
where quantization errors either just scale softmax weights by ~0.4% or
average out over the 2048-deep o_proj contraction (~3e-3 rel err total,
measured by host emulation). fp16 would fit the Q/K ranges too but streams
~20% slower than bf16/fp32r on the PE (measured). PSUM stays fp32.
"""

import sys

for _p in ("/opt/trn_rl_repo", "/root/.axon_site/_ro/trn_rl_repo"):
    if _p not in sys.path:
        sys.path.insert(0, _p)

import numpy as np

B = 2
S = 2048
HID = 2048
NH = 16
NKV = 2
D = 128
NQ = 512          # query rows per core (two 256-row strips)
QW = 256          # strip width
N_CORES = 8
SM_SCALE = 1.0 / np.sqrt(np.float32(D))

_BUILD_CACHE = {}


def _round_fp32r(a):
    """Round-to-nearest-even to 12 explicit mantissa bits (fp32r)."""
    u = np.ascontiguousarray(a, np.float32).view(np.uint32)
    low = u & np.uint32(0xFFF)
    up = (u & np.uint32(0xFFFFF000)) + np.uint32(0x1000)
    half = low == np.uint32(0x800)
    rnd = np.where(low > 0x800, up,
                   np.where(half & ((u & np.uint32(0x1000)) != 0), up,
                            u & np.uint32(0xFFFFF000)))
    expmask = (u & np.uint32(0x7F800000)) == np.uint32(0x7F800000)
    rnd = np.where(expmask, u, rnd)
    return rnd.view(np.float32)


def _build_nc(mm="mix"):
    import contextlib
    import concourse.bass as bass
    import concourse.tile as tile
    from concourse import bacc, mybir

    F32 = mybir.dt.float32
    if mm == "mix":
        DT_X = mybir.dt.float32r  # x/w/q/k/cos/sin path (score exponents)
        DT_E = mybir.dt.bfloat16  # exp-output/V/mask path
        DT_O = mybir.dt.bfloat16  # o_proj operands
    elif mm == "f32r":
        DT_X = DT_E = DT_O = mybir.dt.float32r
    else:
        DT_X = DT_E = DT_O = F32

    nc = bacc.Bacc(target_bir_lowering=False, debug=False)

    def param(name, shape, dt):
        return nc.declare_dram_parameter(name, list(shape), dt,
                                         isOutput=False)[:]

    xT = param("xT", [HID, S], DT_X)
    xq_d = param("xq", [HID, NQ], DT_X)
    wqT = param("wqT", [HID, HID], DT_X)
    wkvT = param("wkvT", [HID, 2 * NKV * D], DT_X)   # [wk | wv]
    woT = param("woT", [HID, HID], DT_O)
    bqT_d = param("bqT", [D, NH], F32)
    bkT_d = param("bkT", [D, NKV], F32)
    bv_d = param("bv", [1, NKV * D], DT_X)
    cosT_d = param("cosT", [D, S], DT_X)
    sinT_d = param("sinT", [D, S], DT_X)
    cq_d = param("cosTq", [D, NQ], DT_X)
    sq_d = param("sinTq", [D, NQ], DT_X)
    # exp(mask) pre-packed into SBUF layout (see _host_prep):
    #   maskFA: phase-1 A-slot masks, 4 groups x [128, 2, 256]
    #   maskB:  phase-2 masks (cols duplicated per head pair),
    #           4 groups x [128, 2, 512]
    maskFA_d = param("maskFA", [128, 4 * 2 * QW], DT_E)
    maskB_d = param("maskB", [128, 4 * 2 * NQ], DT_E)
    out_d = nc.declare_dram_parameter("out", [NQ, HID], F32, isOutput=True)[:]

    HC = HID // 128   # 16 contraction chunks
    KT = S // 128     # 16 key tiles
    QS = NQ // 128    # 4 query sub-tiles

    Exp = mybir.ActivationFunctionType.Exp
    Ident = mybir.ActivationFunctionType.Identity

    lp = (nc.allow_low_precision(reason="low-precision matmul operands; "
                                 "psum stays f32")
          if mm in ("mix", "f32r") else contextlib.nullcontext())
    with lp, tile.TileContext(nc) as tc:
        with tc.tile_pool(name="const", bufs=1) as cst, \
             tc.tile_pool(name="maskp", bufs=1) as maskp, \
             tc.tile_pool(name="kvp", bufs=1) as kvp:

            ones_row = cst.tile([1, 128], DT_X, name="ones_row")
            ones_sq = cst.tile([128, 128], DT_E, name="ones_sq")
            ones_f32 = cst.tile([128, 128], F32, name="ones_f32")
            nc.vector.memset(ones_f32, 1.0)
            nc.vector.tensor_copy(ones_row, ones_f32[0:1, :])
            nc.vector.tensor_copy(ones_sq, ones_f32)
            bqT = cst.tile([D, NH], F32, name="bqT")
            bkT = cst.tile([D, NKV], F32, name="bkT")
            bvr = cst.tile([1, NKV * D], DT_X, name="bvr")
            nc.sync.dma_start(bqT, bqT_d)
            nc.sync.dma_start(bkT, bkT_d)
            nc.sync.dma_start(bvr, bv_d)

            # exp(mask) tiles, resident through attention
            maskFA_sb = [maskp.tile([128, 2, QW], DT_E, name=f"mFA{g}")
                         for g in range(4)]
            maskB_sb = [maskp.tile([128, 2, NQ], DT_E, name=f"mB{g}")
                        for g in range(4)]

            # this core's query columns of xT (host-gathered), resident for
            # the Q projection
            xq_sb = [kvp.tile([128, NQ], DT_X, name=f"xq{c}")
                     for c in range(HC)]
            # persistent K^T [d, t] per kv head; V [t, d] per token tile
            kT_sb = [kvp.tile([128, S], DT_X, name=f"kT{g}")
                     for g in range(NKV)]
            v_sb = [kvp.tile([128, NKV * D], DT_E, name=f"v{t}")
                    for t in range(KT)]

            # ---------------- P1a: K/V projection over all tokens ----------
            # Sync queue carries the x/wkv/cos-sin stream; the Scalar queue
            # carries the wq/xq stream for the later phases.
            with tc.tile_pool(name="p1", bufs=1) as p1, \
                 tc.tile_pool(name="p1s", bufs=3) as p1s, \
                 tc.tile_pool(name="p1ps", bufs=1, space="PSUM") as p1ps:
                wkv_sb = [p1.tile([128, 2 * NKV * D], DT_X, name=f"wkv{c}")
                          for c in range(HC)]

                for tp in range(2):          # token half [tp*1024, +1024)
                    xts = [p1s.tile([128, 2, NQ], DT_X, name=f"xt{c}",
                                    bufs=1) for c in range(HC)]
                    for c in range(HC):
                        if tp == 0:
                            nc.sync.dma_start(wkv_sb[c],
                                              wkvT[c * 128:(c + 1) * 128, :])
                        nc.sync.dma_start(
                            xts[c],
                            xT[c * 128:(c + 1) * 128,
                               tp * 2 * NQ:(tp + 1) * 2 * NQ].rearrange(
                                "p (a q) -> p a q", a=2))
                        # xq prefetch on the otherwise-idle Scalar queue
                        if tp == 0:
                            nc.scalar.dma_start(
                                xq_sb[c], xq_d[c * 128:(c + 1) * 128, :])

                    csb = p1s.tile([128, 2, NQ], DT_X, name="csb", bufs=1)
                    ssb = p1s.tile([128, 2, NQ], DT_X, name="ssb", bufs=1)
                    nc.sync.dma_start(
                        csb, cosT_d[:, tp * 2 * NQ:(tp + 1) * 2 * NQ]
                        .rearrange("p (a q) -> p a q", a=2))
                    nc.sync.dma_start(
                        ssb, sinT_d[:, tp * 2 * NQ:(tp + 1) * 2 * NQ]
                        .rearrange("p (a q) -> p a q", a=2))

                    for half in range(2):
                        tch = 2 * tp + half
                        kps = [p1ps.tile([128, NQ], F32, name=f"kps{g}",
                                         bufs=2) for g in range(NKV)]
                        vps = [p1ps.tile([128, NKV * D], F32,
                                         name=f"vps{s_}", bufs=1)
                               for s_ in range(4)]
                        for c in range(HC):
                            xt = xts[c][:, half, :]
                            for g in range(NKV):
                                nc.tensor.matmul(
                                    kps[g], wkv_sb[c][:, g * D:(g + 1) * D],
                                    xt, start=(c == 0), stop=(c == HC - 1))
                            for s_ in range(4):
                                nc.tensor.matmul(
                                    vps[s_], xt[:, s_ * 128:(s_ + 1) * 128],
                                    wkv_sb[c][:, NKV * D:2 * NKV * D],
                                    start=(c == 0), stop=False)
                        # V bias via K=1 ones matmul, then evacuate
                        for s_ in range(4):
                            nc.tensor.matmul(vps[s_], ones_row, bvr,
                                             start=False, stop=True)
                            nc.vector.tensor_copy(v_sb[tch * 4 + s_],
                                                  vps[s_])
                        # K bias + rope -> kT_sb
                        tsl = slice(tch * NQ, (tch + 1) * NQ)
                        for g in range(NKV):
                            kb = p1s.tile([128, NQ], DT_X, name="kb")
                            nc.scalar.activation(kb, kps[g], Ident,
                                                 bias=bkT[:, g:g + 1])
                            ke = kT_sb[g][:, tsl]
                            shuf = p1s.tile([128, NQ], DT_X, name="shuf")
                            nc.sync.dma_start(shuf[0:64, :], kb[64:128, :])
                            nc.sync.dma_start(shuf[64:128, :], kb[0:64, :])
                            nc.vector.tensor_mul(ke, kb, csb[:, half, :])
                            nc.vector.tensor_mul(shuf, shuf,
                                                 ssb[:, half, :])
                            nc.vector.tensor_add(ke, ke, shuf)

            # -------- P1b + P2: Q proj interleaved with attention ----------
            with tc.tile_pool(name="ap", bufs=1) as ap:
                a_sb = [ap.tile([128, NQ], DT_O, name=f"a{h}")
                        for h in range(NH)]
                with tc.tile_pool(name="p2", bufs=1) as p2, \
                     tc.tile_pool(name="p2s", bufs=2) as p2s, \
                     tc.tile_pool(name="p2w", bufs=12) as p2w, \
                     tc.tile_pool(name="qtp", bufs=2) as qtp, \
                     tc.tile_pool(name="att", bufs=4) as att, \
                     tc.tile_pool(name="atts", bufs=1) as atts:
                    cq = p2.tile([D, NQ], DT_X, name="cq")
                    sq = p2.tile([D, NQ], DT_X, name="sq")
                    nc.sync.dma_start(cq, cq_d)
                    nc.sync.dma_start(sq, sq_d)

                    for hg in range(4):
                        qT_sb = {}
                        with tc.tile_pool(name=f"qps{hg}", bufs=1,
                                          space="PSUM") as p2ps:
                            qps = [p2ps.tile([128, NQ], F32, name=f"qps{j}",
                                             bufs=1) for j in range(4)]
                            for c in range(HC):
                                wq = p2w.tile([128, NQ], DT_X, name="wq")
                                nc.scalar.dma_start(
                                    wq, wqT[c * 128:(c + 1) * 128,
                                            hg * NQ:(hg + 1) * NQ])
                                for j in range(4):
                                    nc.tensor.matmul(
                                        qps[j], wq[:, j * 128:(j + 1) * 128],
                                        xq_sb[c], start=(c == 0),
                                        stop=(c == HC - 1))
                            for j in range(4):
                                h = hg * 4 + j
                                qT_sb[h] = qtp.tile([128, NQ], DT_X,
                                                    name=f"qT{j}")
                                qb = p2s.tile([128, NQ], DT_X, name="qb")
                                nc.scalar.activation(qb, qps[j], Ident,
                                                     bias=bqT[:, h:h + 1])
                                qe = qT_sb[h]
                                shufq = p2s.tile([128, NQ], DT_X,
                                                 name="shufq")
                                nc.sync.dma_start(shufq[0:64, :],
                                                  qb[64:128, :])
                                nc.sync.dma_start(shufq[64:128, :],
                                                  qb[0:64, :])
                                nc.vector.tensor_mul(qe, qb, cq)
                                nc.vector.tensor_mul(shufq, shufq, sq)
                                nc.vector.tensor_add(qe, qe, shufq)

                        if hg == 0:
                            for g_ in range(4):
                                nc.sync.dma_start(
                                    maskFA_sb[g_],
                                    maskFA_d[:, g_ * 2 * QW:
                                             (g_ + 1) * 2 * QW].rearrange(
                                        "p (a q) -> p a q", a=2))
                                nc.sync.dma_start(
                                    maskB_sb[g_],
                                    maskB_d[:, g_ * 2 * NQ:
                                            (g_ + 1) * 2 * NQ].rearrange(
                                        "p (a q) -> p a q", a=2))

                        # B-slot halves of the two head pairs, contiguous
                        # [h0B | h1B] for phase-2 N=512 matmuls
                        qBp = [qtp.tile([128, NQ], DT_X, name=f"qBp{p_}")
                               for p_ in range(2)]
                        for p_ in range(2):
                            for i_ in range(2):
                                h = hg * 4 + p_ * 2 + i_
                                nc.vector.tensor_copy(
                                    qBp[p_][:, i_ * QW:(i_ + 1) * QW],
                                    qT_sb[h][:, QW:NQ])

                        with tc.tile_pool(name=f"attps{hg}", bufs=1,
                                          space="PSUM") as attps:
                            def phase1(h):
                                """Key tiles 0..7, both slots (N=512)."""
                                g = h // (NH // NKV)
                                ops = attps.tile([128, NQ], F32, name="ops",
                                                 bufs=1)
                                stats = attps.tile([128, NQ], F32,
                                                   name="stats", bufs=1)
                                for grp in range(4):
                                    sps = attps.tile([128, 2, NQ], F32,
                                                     name="sps", bufs=2)
                                    ebuf = att.tile([128, 2, NQ], DT_E,
                                                    name="ebuf")
                                    for j2 in range(2):
                                        kt = 2 * grp + j2
                                        nc.tensor.matmul(
                                            sps[:, j2, :],
                                            kT_sb[g][:, kt * 128:
                                                     (kt + 1) * 128],
                                            qT_sb[h], start=True, stop=True)
                                    nc.scalar.activation(
                                        ebuf.rearrange("p a b -> p (a b)"),
                                        sps.rearrange("p a b -> p (a b)"),
                                        Exp, scale=float(SM_SCALE))
                                    # causal mask on A-slot columns only
                                    for j2 in range(2):
                                        nc.vector.tensor_mul(
                                            ebuf[:, j2, 0:QW],
                                            ebuf[:, j2, 0:QW],
                                            maskFA_sb[grp][:, j2, :])
                                    for j2 in range(2):
                                        kt = 2 * grp + j2
                                        nc.tensor.matmul(
                                            stats, ones_sq, ebuf[:, j2, :],
                                            start=(kt == 0),
                                            stop=(kt == KT // 2 - 1))
                                        nc.tensor.matmul(
                                            ops,
                                            v_sb[kt][:, g * D:(g + 1) * D],
                                            ebuf[:, j2, :],
                                            start=(kt == 0),
                                            stop=(kt == KT // 2 - 1))
                                return ops, stats

                            def phase2(p_):
                                """Key tiles 8..15, paired B-slots (N=512)."""
                                h0 = hg * 4 + p_ * 2
                                g = h0 // (NH // NKV)
                                opsB = attps.tile([128, NQ], F32,
                                                  name="opsB", bufs=1)
                                statsB = attps.tile([128, NQ], F32,
                                                    name="statsB", bufs=1)
                                for grp in range(4):
                                    sps = attps.tile([128, 2, NQ], F32,
                                                     name="sps", bufs=2)
                                    ebuf = att.tile([128, 2, NQ], DT_E,
                                                    name="ebuf")
                                    for j2 in range(2):
                                        kt = 8 + 2 * grp + j2
                                        nc.tensor.matmul(
                                            sps[:, j2, :],
                                            kT_sb[g][:, kt * 128:
                                                     (kt + 1) * 128],
                                            qBp[p_], start=True, stop=True)
                                    nc.scalar.activation(
                                        ebuf.rearrange("p a b -> p (a b)"),
                                        sps.rearrange("p a b -> p (a b)"),
                                        Exp, scale=float(SM_SCALE))
                                    nc.vector.tensor_mul(
                                        ebuf.rearrange("p a b -> p (a b)"),
                                        ebuf.rearrange("p a b -> p (a b)"),
                                        maskB_sb[grp].rearrange(
                                            "p a b -> p (a b)"))
                                    for j2 in range(2):
                                        kt = 8 + 2 * grp + j2
                                        nc.tensor.matmul(
                                            statsB, ones_sq, ebuf[:, j2, :],
                                            start=(kt == 8),
                                            stop=(kt == KT - 1))
                                        nc.tensor.matmul(
                                            opsB,
                                            v_sb[kt][:, g * D:(g + 1) * D],
                                            ebuf[:, j2, :],
                                            start=(kt == 8),
                                            stop=(kt == KT - 1))
                                return opsB, statsB

                            def finish(h, ph1, ph2, i_):
                                """Combine phase sums, normalize -> a_sb."""
                                ops, stats = ph1
                                opsB, statsB = ph2
                                # avoid two-PSUM-operand DVE ops: copy to
                                # SBUF first, then add the phase-2 PSUM in
                                bsl = slice(i_ * QW, (i_ + 1) * QW)
                                ssum = atts.tile([128, NQ], F32,
                                                 name="ssum")
                                nc.vector.tensor_copy(ssum, stats)
                                nc.vector.tensor_add(ssum[:, QW:NQ],
                                                     ssum[:, QW:NQ],
                                                     statsB[:, bsl])
                                osum = atts.tile([128, NQ], F32,
                                                 name="osum")
                                nc.vector.tensor_copy(osum, ops)
                                nc.vector.tensor_add(osum[:, QW:NQ],
                                                     osum[:, QW:NQ],
                                                     opsB[:, bsl])
                                recip = atts.tile([128, NQ], F32,
                                                  name="recip")
                                nc.vector.reciprocal_approx_fast(
                                    out=recip, in_=ssum)
                                nc.vector.tensor_mul(a_sb[h], osum, recip)

                            # ops/stats are bufs=1: finish(h0) must free
                            # them before phase1(h0+1) reallocates
                            for p_ in range(2):
                                h0 = hg * 4 + p_ * 2
                                r0 = phase1(h0)
                                r2 = phase2(p_)
                                finish(h0, r0, r2, 0)
                                r1 = phase1(h0 + 1)
                                finish(h0 + 1, r1, r2, 1)

                # ------------- P3: o_proj ------------------------------
                with tc.tile_pool(name="wop", bufs=1) as wop, \
                     tc.tile_pool(name="wos", bufs=3) as wos, \
                     tc.tile_pool(name="wops", bufs=1, space="PSUM") as wops:
                    for ec in range(4):
                        wo_t = [wop.tile([128, NQ], DT_O, name=f"wo{h}",
                                         bufs=2) for h in range(NH)]
                        for h in range(NH):
                            nc.sync.dma_start(
                                wo_t[h], woT[h * 128:(h + 1) * 128,
                                             ec * NQ:(ec + 1) * NQ])
                        for qs_ in range(QS):
                            opo = wops.tile([128, NQ], F32, name="opo",
                                            bufs=3)
                            for h in range(NH):
                                nc.tensor.matmul(
                                    opo,
                                    a_sb[h][:, qs_ * 128:(qs_ + 1) * 128],
                                    wo_t[h], start=(h == 0),
                                    stop=(h == NH - 1))
                            osb = wos.tile([128, NQ], F32, name="osb")
                            nc.vector.tensor_copy(osb, opo)
                            nc.sync.dma_start(
                                out_d[qs_ * 128:(qs_ + 1) * 128,
                                      ec * NQ:(ec + 1) * NQ], osb)
    return nc


def get_nc(mm="mix"):
    if mm not in _BUILD_CACHE:
        nc = _build_nc(mm)
        nc.finalize()
        _BUILD_CACHE[mm] = nc
    return _BUILD_CACHE[mm]


_MROPE_SECTION = [16, 24, 24]
_STREAM_IDX = np.concatenate(
    [np.full(n, i % 3, np.int64)
     for i, n in enumerate(_MROPE_SECTION * 2)])  # [128]


def _host_prep(hidden_states, cos, sin, attention_mask, Wq, bq, Wk, bk, Wv,
               bv, Wo, mm="mix"):
    f = np.float32
    if mm == "mix":
        import ml_dtypes

        rnd = _round_fp32r

        def rnd_e(a):
            return np.ascontiguousarray(a, f).astype(ml_dtypes.bfloat16)
        rnd_o = rnd_e
    elif mm == "f32r":
        rnd = rnd_e = rnd_o = _round_fp32r
    else:
        def rnd(a):
            return np.ascontiguousarray(a, f)
        rnd_e = rnd_o = rnd
    hs = np.asarray(hidden_states, f)
    cos = np.asarray(cos, f)
    sin = np.asarray(sin, f)
    mask = np.asarray(attention_mask, f)
    ar = np.arange(D)

    shared = {
        "wqT": rnd(np.asarray(Wq, f).T),
        "wkvT": rnd(np.concatenate([np.asarray(Wk, f).T,
                                    np.asarray(Wv, f).T], axis=1)),
        "woT": rnd_o(np.asarray(Wo, f).T),
        "bqT": np.ascontiguousarray(np.asarray(bq, f).reshape(NH, D).T),
        "bkT": np.ascontiguousarray(np.asarray(bk, f).reshape(NKV, D).T),
        "bv": rnd(np.asarray(bv, f).reshape(1, NKV * D)),
    }

    per_batch = []
    for b in range(B):
        xT = hs[b].T
        cosT = cos[_STREAM_IDX, b, :, ar].copy()  # [128, S]
        sinT = sin[_STREAM_IDX, b, :, ar].copy()
        sinT[0:64, :] *= -1.0   # rotate_half sign folded into sin
        maskT = np.exp(mask[b, 0].T.astype(np.float64)).astype(np.float32)
        per_batch.append((xT, cosT, sinT, maskT))

    in_maps = []
    for c in range(N_CORES):
        b, j = divmod(c, N_CORES // B)
        xT, cosT, sinT, maskT = per_batch[b]
        qcols = np.concatenate([np.arange(j * QW, (j + 1) * QW),
                                np.arange((7 - j) * QW, (8 - j) * QW)])
        m = dict(shared)
        m["xT"] = rnd(xT)
        m["cosT"] = rnd(cosT)
        m["sinT"] = rnd(sinT)
        m["xq"] = rnd(xT[:, qcols])
        m["cosTq"] = rnd(cosT[:, qcols])
        m["sinTq"] = rnd(sinT[:, qcols])
        # phase-1 A-slot masks: keys 0..1023 x strip-j queries,
        # packed [p, grp, a, q] -> [128, 4*2*256]
        emA = maskT[0:S // 2, qcols[:QW]]            # [1024, 256]
        m["maskFA"] = rnd_e(
            emA.reshape(4, 2, 128, QW).transpose(2, 0, 1, 3).reshape(
                128, 4 * 2 * QW))
        # phase-2 masks: keys 1024..2047 x strip-(7-j) queries, duplicated
        # for the head pair, packed [p, grp, a, (dup q)] -> [128, 4*2*512]
        emB = maskT[S // 2:S, qcols[QW:]]            # [1024, 256]
        emB = emB.reshape(4, 2, 128, QW).transpose(2, 0, 1, 3)  # [128,4,2,q]
        emB = np.concatenate([emB[..., None, :], emB[..., None, :]],
                             axis=3)                 # [128, 4, 2, 2, 256]
        m["maskB"] = rnd_e(emB.reshape(128, 4 * 2 * NQ))
        in_maps.append(m)
    return in_maps


def kernel(hidden_states, cos, sin, attention_mask, Wq, bq, Wk, bk, Wv, bv,
           Wo, _trace=False, _mm="mix"):
    from concourse.bass_utils import run_bass_kernel_spmd

    in_maps = _host_prep(hidden_states, cos, sin, attention_mask, Wq, bq, Wk,
                         bk, Wv, bv, Wo, mm=_mm)
    nc = get_nc(_mm)
    res = run_bass_kernel_spmd(nc, in_maps, list(range(N_CORES)),
                               trace=_trace)
    out = np.empty((B, S, HID), np.float32)
    for c in range(N_CORES):
        b, j = divmod(c, N_CORES // B)
        qcols = np.concatenate([np.arange(j * QW, (j + 1) * QW),
                                np.arange((7 - j) * QW, (8 - j) * QW)])
        out[b, qcols, :] = res.results[c]["out"]
    kernel._last_results = res
    return out

